# revision 12
# baseline (speedup 1.0000x reference)
"""Trainium2 Bass kernel for EquiDeformableAttn (triplane deformable attention).

Strategy (per core = one batch element, data-parallel over B=8):
  - Triplanes relayouted on host to row-pair form pt2[y*W+x] = [plane[:, y, x],
    plane[:, y+1, x]] (2C floats per entry). One dma_gather element (2KB,
    spanning entries (y0,x0) and (y0,x0+1)) holds all 4 bilinear corners
    [f00|f10|f01|f11].
  - All per-point tensors use n-rows layout [n%128 partitions, n//128 cols].
  - Gathers use prepare_only + trigger_dma so Q7 descriptor generation and the
    SDMA transfers pipeline instead of serializing (the transfer completion is
    observed by consumers via per-queue DMA semaphores, Tile-managed).
  - The int16 gather-index layout ([s%16 partition, s//16 free], replicated to
    128 partitions) is built with PE selection matmuls (identity-slice lhsT)
    + one strided copy, instead of per-element 2-byte DMA descriptors.
  - Weight foldings eliminate large matmuls:
      qh = feat @ (Wq @ Wk.T) * sqrt(C) + (Wk @ bq) * sqrt(C)
      out = (sum_j attn_j aux_feat_j) @ (Wv @ Wout) + (bv @ Wout + bout) + feat
  - Everything on the similarity path stays f32: 16-bit features/weights
    perturb sim (scale ~150) enough to flip near-tie softmax weights.
"""

import numpy as np

C = 128
S = 8
H = 64
W = 64
SQRTC = float(C) ** 0.5

_CACHE = {}


def _build(NQ, n_devices=8):
    import concourse.bass as bass
    import concourse.bacc as bacc
    import concourse.tile as tile
    import concourse.mybir as mybir
    from concourse import library_config

    F32, I32, I16 = mybir.dt.float32, mybir.dt.int32, mybir.dt.int16
    AF = mybir.ActivationFunctionType
    Alu = mybir.AluOpType
    AX = mybir.AxisListType.X

    G = NQ // 128                  # n-rows free columns
    CHQA = min(1024, NQ)           # phase-A gather chunk (queries)
    NCHA = NQ // CHQA
    PLANES = [(0, 2), (0, 1), (1, 2)]   # (x-axis, y-axis) per plane

    nc = bacc.Bacc("TRN2", target_bir_lowering=False, debug=False,
                   num_devices=n_devices, num_swdge_queues=4)

    qp_d = nc.dram_tensor("qp", [128, G * 3], F32, kind="ExternalInput")
    pt2_d = [nc.dram_tensor(f"pt2_{i}", [H * W, 2 * C], F32,
                            kind="ExternalInput") for i in range(3)]
    wq_d = nc.dram_tensor("Wq", [C, C], F32, kind="ExternalInput")
    wk_d = nc.dram_tensor("Wk", [C, C], F32, kind="ExternalInput")
    wv_d = nc.dram_tensor("Wv", [C, C], F32, kind="ExternalInput")
    wout_d = nc.dram_tensor("Wout", [C, C], F32, kind="ExternalInput")
    woff_d = nc.dram_tensor("Woff", [C, 3 * S], F32, kind="ExternalInput")
    bq_d = nc.dram_tensor("bq", [C, 1], F32, kind="ExternalInput")
    bv_d = nc.dram_tensor("bv", [C, 1], F32, kind="ExternalInput")
    bout_d = nc.dram_tensor("bout", [1, C], F32, kind="ExternalInput")
    boff_d = nc.dram_tensor("boff", [3 * S, 1], F32, kind="ExternalInput")
    ident_d = nc.dram_tensor("ident", [128, 128], F32, kind="ExternalInput")
    out_d = nc.dram_tensor("out", [128, G * C], F32, kind="ExternalOutput")

    with tile.TileContext(nc) as tc:
        nc.gpsimd.load_library(library_config.mlp)
        gsem = [nc.alloc_semaphore(f"gsem{q}") for q in range(4)]

        with tc.tile_pool(name="const", bufs=1) as cp, \
             tc.tile_pool(name="gat", bufs=2) as gp, \
             tc.tile_pool(name="wrk", bufs=1) as wp, \
             tc.tile_pool(name="psum", bufs=4, space="PSUM") as pp, \
             tc.tile_pool(name="psumF", bufs=1, space="PSUM") as pf:

            ident = cp.tile([128, 128], F32)
            nc.sync.dma_start(out=ident[:], in_=ident_d[:])
            c315 = cp.tile([128, 1], F32)
            nc.vector.memset(c315[:], 31.5)

            wq = cp.tile([C, C], F32); nc.sync.dma_start(out=wq[:], in_=wq_d[:])
            wk = cp.tile([C, C], F32); nc.sync.dma_start(out=wk[:], in_=wk_d[:])
            wv = cp.tile([C, C], F32); nc.sync.dma_start(out=wv[:], in_=wv_d[:])
            wout = cp.tile([C, C], F32); nc.sync.dma_start(out=wout[:], in_=wout_d[:])
            woff = cp.tile([C, 3 * S], F32); nc.sync.dma_start(out=woff[:], in_=woff_d[:])
            bq = cp.tile([C, 1], F32); nc.sync.dma_start(out=bq[:], in_=bq_d[:])
            bv = cp.tile([C, 1], F32); nc.sync.dma_start(out=bv[:], in_=bv_d[:])
            bout = cp.tile([1, C], F32); nc.sync.dma_start(out=bout[:], in_=bout_d[:])
            boff = cp.tile([3 * S, 1], F32); nc.sync.dma_start(out=boff[:], in_=boff_d[:])

            def transpose_sb(dst_ap, src_ap, kdim):
                ps = pp.tile([128, 512], F32, tag="ps")
                p = ps[: src_ap.shape[1], : src_ap.shape[0]]
                nc.tensor.transpose(p, src_ap, ident[:kdim, :kdim])
                nc.scalar.copy(out=dst_ap, in_=p)

            wqT = cp.tile([C, C], F32); transpose_sb(wqT[:], wq[:], 128)
            wkT = cp.tile([C, C], F32); transpose_sb(wkT[:], wk[:], 128)
            wvT = cp.tile([C, C], F32); transpose_sb(wvT[:], wv[:], 128)

            # AQK = Wq @ Wk.T * sqrt(C);  bqk = Wk @ bq * sqrt(C)
            ps1 = pp.tile([128, 512], F32, tag="ps")
            nc.tensor.matmul(ps1[:, :128], lhsT=wqT[:], rhs=wkT[:], start=True, stop=True)
            aqk = cp.tile([C, C], F32)
            nc.vector.tensor_scalar(out=aqk[:], in0=ps1[:, :128], scalar1=SQRTC,
                                    scalar2=None, op0=Alu.mult)
            ps2 = pp.tile([128, 512], F32, tag="ps")
            nc.tensor.matmul(ps2[:, :1], lhsT=wkT[:], rhs=bq[:], start=True, stop=True)
            bqk = cp.tile([C, 1], F32)
            nc.vector.tensor_scalar(out=bqk[:], in0=ps2[:, :1], scalar1=SQRTC,
                                    scalar2=None, op0=Alu.mult)
            # Wvo = Wv @ Wout; bfull broadcast
            ps3 = pp.tile([128, 512], F32, tag="ps")
            nc.tensor.matmul(ps3[:, :128], lhsT=wvT[:], rhs=wout[:], start=True, stop=True)
            wvo = cp.tile([C, C], F32)
            nc.vector.tensor_copy(out=wvo[:], in_=ps3[:, :128])
            ps4 = pp.tile([128, 512], F32, tag="ps")
            nc.tensor.matmul(ps4[:1, :128], lhsT=bv[:], rhs=wout[:], start=True, stop=True)
            bfrow = cp.tile([1, C], F32)
            nc.vector.tensor_tensor(out=bfrow[:], in0=ps4[:1, :128], in1=bout[:], op=Alu.add)
            ones_row = cp.tile([1, 128], F32)
            nc.vector.memset(ones_row[:], 1.0)
            ps5 = pp.tile([128, 512], F32, tag="ps")
            nc.tensor.matmul(ps5[:, :128], lhsT=ones_row[:], rhs=bfrow[:], start=True, stop=True)
            bfull = cp.tile([128, C], F32)
            nc.vector.tensor_copy(out=bfull[:], in_=ps5[:, :128])

            qp_rows = cp.tile([128, G * 3], F32)
            nc.sync.dma_start(out=qp_rows[:], in_=qp_d[:])

            # ---- coords -> (wrapped idx16, bilinear tap weights) ----
            # wrap layout: idx of gather-slot s at [s%16, s//16], replicated to
            # all 128 partitions; slot s = col*128 + p (n-rows order).
            def coord_phase(src_rows, gcols, jd, name):
                wraps, w4s = [], []
                nf = gcols * jd
                for pl, (axx, axy) in enumerate(PLANES):
                    def floorclip(coord_ap, tg):
                        # xa = clip((c+1)*31.5, 0, 63); x0 = floor(xa) via
                        # round-nearest(xa - 0.5) (exact-integer xa only at
                        # the clip bounds, where the half-even rounding still
                        # yields a correct (x0, wx) pair); wx = xa - x0.
                        xa = wp.tile([128, nf], F32, tag=f"ca{tg}")
                        nc.scalar.activation(out=xa[:], in_=coord_ap,
                                             func=AF.Relu, bias=c315[:], scale=31.5)
                        xs = wp.tile([128, nf], F32, tag=f"cs{tg}")
                        nc.vector.tensor_scalar(out=xs[:], in0=xa[:], scalar1=63.0,
                                                scalar2=0.5, op0=Alu.min,
                                                op1=Alu.subtract)
                        xi = wp.tile([128, nf], I32, tag=f"ci{tg}")
                        nc.vector.tensor_copy(out=xi[:], in_=xs[:])  # round-nearest
                        x0 = wp.tile([128, nf], F32, tag=f"c0{tg}")
                        nc.vector.tensor_copy(out=x0[:], in_=xi[:])
                        wx = wp.tile([128, nf], F32, tag=f"cw{tg}")
                        nc.vector.scalar_tensor_tensor(out=wx[:], in0=xa[:], scalar=63.0,
                                                       in1=x0[:], op0=Alu.min,
                                                       op1=Alu.subtract)
                        return x0, wx

                    xs_ = src_rows[:, :, :, axx].rearrange("p g j -> p (g j)")
                    ys_ = src_rows[:, :, :, axy].rearrange("p g j -> p (g j)")
                    x0, wx = floorclip(xs_, "x")
                    y0, wy = floorclip(ys_, "y")

                    hwf = wp.tile([128, nf], F32, tag="chw")
                    nc.vector.scalar_tensor_tensor(out=hwf[:], in0=y0[:], scalar=float(W),
                                                   in1=x0[:], op0=Alu.mult, op1=Alu.add)

                    omx = wp.tile([128, nf], F32, tag="cox")
                    nc.vector.tensor_scalar(out=omx[:], in0=wx[:], scalar1=-1.0,
                                            scalar2=1.0, op0=Alu.mult, op1=Alu.add)
                    omy = wp.tile([128, nf], F32, tag="coy")
                    nc.vector.tensor_scalar(out=omy[:], in0=wy[:], scalar1=-1.0,
                                            scalar2=1.0, op0=Alu.mult, op1=Alu.add)
                    w4 = wp.tile([128, nf * 4], F32, tag=f"{name}w4{pl}")
                    w4v = w4[:].rearrange("p (f t) -> p f t", t=4)
                    nc.vector.tensor_tensor(out=w4v[:, :, 0], in0=omx[:], in1=omy[:], op=Alu.mult)
                    nc.vector.tensor_tensor(out=w4v[:, :, 1], in0=omx[:], in1=wy[:], op=Alu.mult)
                    nc.vector.tensor_tensor(out=w4v[:, :, 2], in0=wx[:], in1=omy[:], op=Alu.mult)
                    nc.vector.tensor_tensor(out=w4v[:, :, 3], in0=wx[:], in1=wy[:], op=Alu.mult)

                    # Fold [128, nf] -> [16, nf*8] (idx s at [s%16, s//16]):
                    # psF[:16, q*nf + f] = hwf[16q + a, f] via 8 identity-slice
                    # matmuls, then one strided cast-copy into (f, q) order.
                    wrap = wp.tile([128, nf * 8], I16, tag=f"{name}wr{pl}")
                    psF = pf.tile([128, 8 * nf], F32, tag="fold")
                    for q in range(8):
                        nc.tensor.matmul(psF[:16, q * nf:(q + 1) * nf],
                                         lhsT=ident[:, 16 * q:16 * (q + 1)],
                                         rhs=hwf[:], start=True, stop=True)
                    nc.vector.tensor_copy(
                        out=wrap[:16, :].rearrange("p (f q) -> p f q", q=8),
                        in_=psF[:16, :].rearrange("p (q f) -> p f q", q=8))
                    nc.sync.dma_start(out=wrap[16:32, :], in_=wrap[:16, :])
                    nc.sync.dma_start(out=wrap[32:64, :], in_=wrap[:32, :])
                    nc.sync.dma_start(out=wrap[64:128, :], in_=wrap[:64, :])
                    wraps.append(wrap)
                    w4s.append(w4)
                return wraps, w4s

            # ---- gather (prep/trigger) + bilinear combine ----
            def gather_combine(pl, wrap, w4, col0, ncols, acc_ap, first, qn):
                nidx = ncols * 128
                q = qn % 4
                raw = gp.tile([128, 4096], F32, tag="raw", bufs=3)
                gsrc = bass.AP(pt2_d[pl].ap().tensor, 0, [[2 * C, H * W - 1], [1, 4 * C]])
                nc.gpsimd.dma_gather(
                    out_ap=raw[:, : nidx * 4].rearrange("p (g e) -> p g e", e=512),
                    in_ap=gsrc,
                    idxs_ap=wrap[:, col0 * 8: (col0 + ncols) * 8],
                    num_idxs=nidx,
                    num_idxs_reg=nidx,
                    elem_size=512,
                    elem_step=2 * C,
                    queue_num=q,
                )
                rv = raw[:, : nidx * 4].rearrange("p (f t c) -> p f t c", t=4, c=128)
                wv4 = w4[:].rearrange("p (f t) -> p f t", t=4)[:, col0:col0 + ncols, :]
                prod = gp.tile([128, 4096], F32, tag="prod", bufs=1)
                pv = prod[:, : nidx * 4].rearrange("p (f t c) -> p f t c", t=4, c=128)
                # DVE weights the first nd f-columns; ACT (otherwise idle)
                # takes the rest as per-partition-scale copies.
                nd = ncols - 2 if ncols == 8 else ncols
                nc.vector.tensor_tensor(
                    out=pv[:, :nd], in0=rv[:, :nd],
                    in1=wv4[:, :nd].unsqueeze(-1).to_broadcast([128, nd, 4, 128]),
                    op=Alu.mult)
                for f in range(nd, ncols):
                    for t in range(4):
                        k = (col0 + f) * 4 + t
                        nc.scalar.mul(out=pv[:, f, t, :], in_=rv[:, f, t, :],
                                      mul=w4[:, k:k + 1])
                # tree-add the 4 taps: (t0+t1, t2+t3) then sum
                h = gp.tile([128, 2048], F32, tag="h", bufs=1)
                hv = h[:, : nidx * 2].rearrange("p (f a c) -> p f a c", a=2, c=128)
                pvs = prod[:, : nidx * 4].rearrange("p (f a b c) -> p f a b c",
                                                    a=2, b=2, c=128)
                nc.vector.tensor_tensor(out=hv, in0=pvs[:, :, :, 0, :],
                                        in1=pvs[:, :, :, 1, :], op=Alu.add)
                if first:
                    nc.vector.tensor_tensor(out=acc_ap, in0=hv[:, :, 0, :],
                                            in1=hv[:, :, 1, :], op=Alu.add)
                else:
                    tmp = gp.tile([128, 1024], F32, tag="scr", bufs=1)
                    tv = tmp[:, :nidx].rearrange("p (f c) -> p f c", c=128)
                    nc.vector.tensor_tensor(out=tv, in0=hv[:, :, 0, :],
                                            in1=hv[:, :, 1, :], op=Alu.add)
                    nc.vector.tensor_tensor(out=acc_ap, in0=acc_ap, in1=tv, op=Alu.add)

            # ============ PHASE A: features ============
            qpv = qp_rows[:].rearrange("p (g a) -> p g a", a=3).unsqueeze(2)
            wrapsA, w4sA = coord_phase(qpv, G, 1, "A")

            feat_rows = cp.tile([128, G * 128], F32)
            fa = feat_rows[:].rearrange("p (g c) -> p g c", c=128)
            qq = 0
            for ck in range(NCHA):
                gcols = CHQA // 128
                g0 = ck * gcols
                for pl in range(3):
                    gather_combine(pl, wrapsA[pl], w4sA[pl], g0, gcols,
                                   fa[:, g0:g0 + gcols, :], first=(pl == 0), qn=qq)
                    qq += 1

            aux_rows = cp.tile([128, G * 3 * S], F32)
            ar = aux_rows[:].rearrange("p (g j a) -> p g j a", j=S, a=3)
            qh_rows = cp.tile([128, G * 128], F32)
            qhr = qh_rows[:].rearrange("p (g c) -> p g c", c=128)

            with tc.tile_pool(name="phA", bufs=2) as pa:
                NSLC = min(512, NQ)
                GS = NSLC // 128
                for i in range(NQ // NSLC):
                    featT_s = pa.tile([128, NSLC], F32, tag="fT")
                    for t in range(GS):
                        transpose_sb(featT_s[:, 128 * t:128 * (t + 1)],
                                     fa[:, GS * i + t, :], 128)
                    ps = pp.tile([128, NSLC], F32, tag="ps")
                    nc.tensor.matmul(ps[:3 * S, :], lhsT=woff[:], rhs=featT_s[:],
                                     start=True, stop=True)
                    offT_s = pa.tile([3 * S, NSLC], F32, tag="oT")
                    nc.vector.tensor_scalar(out=offT_s[:], in0=ps[:3 * S, :],
                                            scalar1=boff[:], scalar2=None, op0=Alu.add)
                    ps2 = pp.tile([128, NSLC], F32, tag="ps")
                    nc.tensor.matmul(ps2[:], lhsT=aqk[:], rhs=featT_s[:],
                                     start=True, stop=True)
                    qhT_s = pa.tile([C, NSLC], F32, tag="qT")
                    nc.vector.tensor_scalar(out=qhT_s[:], in0=ps2[:],
                                            scalar1=bqk[:], scalar2=None, op0=Alu.add)
                    for t in range(GS):
                        g = GS * i + t
                        transpose_sb(qhr[:, g, :], qhT_s[:, 128 * t:128 * (t + 1)], 128)
                        transpose_sb(aux_rows[:, g * 3 * S:(g + 1) * 3 * S],
                                     offT_s[:, 128 * t:128 * (t + 1)], 3 * S)

            for a in range(3):
                nc.vector.tensor_tensor(
                    out=ar[:, :, :, a], in0=ar[:, :, :, a],
                    in1=qp_rows[:].rearrange("p (g a) -> p g a", a=3)[:, :, a]
                        .unsqueeze(-1).to_broadcast([128, G, S]),
                    op=Alu.add)

            # ============ PHASE B: aux attention ============
            wrapsB, w4sB = coord_phase(ar, G, S, "B")

            wfeat_rows = cp.tile([128, G * 128], F32)
            wfr = wfeat_rows[:].rearrange("p (g c) -> p g c", c=128)

            for ck in range(G):
                g0 = ck
                facc = gp.tile([128, S * 128], F32, tag="facc")
                fav = facc[:].rearrange("p (j c) -> p j c", c=128)
                for pl in range(3):
                    gather_combine(pl, wrapsB[pl], w4sB[pl], g0 * S, S,
                                   fav, first=(pl == 0), qn=qq)
                    qq += 1
                sprod = gp.tile([128, 1024], F32, tag="sprod", bufs=1)
                nc.vector.tensor_tensor(
                    out=sprod[:].rearrange("p (j c) -> p j c", c=128),
                    in0=fav,
                    in1=qhr[:, g0, :].unsqueeze(1).to_broadcast([128, S, 128]),
                    op=Alu.mult)
                sim = gp.tile([128, S], F32, tag="sim")
                nc.vector.tensor_reduce(
                    out=sim[:].unsqueeze(-1),
                    in_=sprod[:].rearrange("p (j c) -> p j c", c=128),
                    axis=AX, op=Alu.add, opt_input=False, opt_output=False)
                mx = gp.tile([128, 1], F32, tag="mx")
                nc.vector.tensor_reduce(out=mx[:], in_=sim[:], axis=AX, op=Alu.max,
                                        opt_input=False, opt_output=False)
                nc.vector.tensor_scalar(out=sim[:], in0=sim[:], scalar1=mx[:],
                                        scalar2=None, op0=Alu.subtract)
                nc.scalar.activation(out=sim[:], in_=sim[:], func=AF.Exp)
                sm = gp.tile([128, 1], F32, tag="sm")
                nc.vector.tensor_reduce(out=sm[:], in_=sim[:], axis=AX, op=Alu.add,
                                        opt_input=False, opt_output=False)
                nc.vector.reciprocal(out=sm[:], in_=sm[:])
                nc.vector.tensor_scalar(out=sim[:], in0=sim[:], scalar1=sm[:],
                                        scalar2=None, op0=Alu.mult)
                wprod = gp.tile([128, 1024], F32, tag="wprod", bufs=2)
                nc.vector.tensor_tensor(
                    out=wprod[:].rearrange("p (j c) -> p j c", c=128),
                    in0=fav,
                    in1=sim[:].unsqueeze(-1).to_broadcast([128, S, 128]),
                    op=Alu.mult)
                nc.vector.tensor_reduce(
                    out=wfr[:, g0, :].unsqueeze(-1),
                    in_=bass.AP(wprod[:].tensor, wprod[:].offset,
                                [wprod[:].ap[0], [1, 128], [128, S]]),
                    axis=AX, op=Alu.add, opt_input=False, opt_output=False)

            # ============ output (in-place into feat_rows) ============
            with tc.tile_pool(name="phO", bufs=2) as po:
                for g in range(G):
                    wfT = po.tile([128, 128], F32, tag="wfT")
                    transpose_sb(wfT[:], wfr[:, g, :], 128)
                    ps = pp.tile([128, 512], F32, tag="ps")
                    nc.tensor.matmul(ps[:, :128], lhsT=wfT[:], rhs=wvo[:],
                                     start=True, stop=True)
                    nc.vector.tensor_tensor(out=fa[:, g, :], in0=ps[:, :128],
                                            in1=fa[:, g, :], op=Alu.add)
                    nc.vector.tensor_tensor(out=fa[:, g, :], in0=fa[:, g, :],
                                            in1=bfull[:], op=Alu.add)
                nc.sync.dma_start(out=out_d[:], in_=feat_rows[:])

    _fix_prep_sems(nc)
    nc.compile()
    return nc


def _fix_prep_sems(nc):
    """Point each gather prep's descriptor completion sem at the Tile DMASW
    lane sem its consumers actually wait on.

    Tile assigns gen_mode==1 SWDGE preps to DMASW lanes round-robin (in
    program order) and generates consumer waits against the lane sems, but
    keeps the author-provided `sem=` as the descriptor's completion sem
    (OnUpdate[0]) — which nothing waits on. Rewrite OnUpdate[0] to the lane
    sem so the SDMA completion increments what the consumers wait for.

    On HW the prep's OnUpdate[0] would fire TWICE per gather: +16 as an
    engine event at prep retire, and +16 from the SDMA descriptors at
    transfer completion (the descriptor increment is hardcoded to 16 in HW;
    only the sem number is encoded). Zero the engine-event increment so only
    the DMA completion bumps the lane sem — exactly the one +16 per gather
    that Tile's generated wait values assume.
    """
    insts = []
    for f in nc.m.functions:
        for blk in f.blocks:
            insts.extend(blk.instructions)
    lane_ids = {}
    lane_names = {}
    for i in insts:
        si = i.sync_info
        if si is None:
            continue
        for w in si.on_wait:
            nm = w.ant_name
            if nm.startswith("DMASW"):
                lane = int(nm.split("_")[0][5:])
                lane_ids[lane] = w.id
                lane_names[lane] = nm
    gathers = [i for i in insts
               if type(i).__name__ == "InstDMAGatherAnt"
               and getattr(i, "gen_mode", 0) == 1]
    if not gathers:
        return
    nlanes = len(lane_ids)
    assert nlanes == 8 and sorted(lane_ids) == list(range(8)), lane_ids
    for gi, g in enumerate(gathers):
        u = g.sync_info.on_update[0]
        assert u.ant_name.startswith("gsem"), (gi, u.ant_name)
        assert u.update_value == 16
        u.id = lane_ids[gi % nlanes]
        u.ant_name = lane_names[gi % nlanes]
        u.update_value = 0


def _prep_core_inputs(qp_b, planes_b, weights, G):
    # qp in n-rows layout: row p holds [qp[g*128+p] for g in range(G)]
    qpr = np.ascontiguousarray(
        qp_b.reshape(G, 128, 3).transpose(1, 0, 2).reshape(128, G * 3), np.float32)
    m = {"qp": qpr}
    for i, pl in enumerate(planes_b):
        t = np.ascontiguousarray(pl.transpose(1, 2, 0), np.float32)  # [H, W, C]
        tn = np.concatenate([t[1:], t[-1:]], axis=0)                 # row y+1
        pt2 = np.stack([t, tn], axis=2).reshape(H * W, 2 * C)
        m[f"pt2_{i}"] = np.ascontiguousarray(pt2)
    m.update(weights)
    return m


def kernel(**inputs):
    from concourse.bass_utils import run_bass_kernel_spmd

    qp = np.asarray(inputs["query_pos"], np.float32)
    B, NQ, _ = qp.shape
    G = NQ // 128
    key = (NQ, B)
    if key not in _CACHE:
        _CACHE[key] = _build(NQ, n_devices=B)
    nc = _CACHE[key]

    weights = dict(
        Wq=np.asarray(inputs["Wq"], np.float32),
        Wk=np.asarray(inputs["Wk"], np.float32),
        Wv=np.asarray(inputs["Wv"], np.float32),
        Wout=np.asarray(inputs["Wout"], np.float32),
        Woff=np.asarray(inputs["Woff"], np.float32),
        bq=np.asarray(inputs["bq"], np.float32).reshape(C, 1),
        bv=np.asarray(inputs["bv"], np.float32).reshape(C, 1),
        bout=np.asarray(inputs["bout"], np.float32).reshape(1, C),
        boff=np.asarray(inputs["boff"], np.float32).reshape(3 * S, 1),
        ident=np.eye(128, dtype=np.float32),
    )
    cxz = np.asarray(inputs["c_xz"], np.float32)
    cxy = np.asarray(inputs["c_xy"], np.float32)
    cyz = np.asarray(inputs["c_yz"], np.float32)

    in_maps = [
        _prep_core_inputs(qp[b], [cxz[b], cxy[b], cyz[b]], weights, G)
        for b in range(B)
    ]
    res = run_bass_kernel_spmd(nc, in_maps, list(range(B)))
    out = np.stack(
        [res.results[b]["out"].reshape(128, G, C).transpose(1, 0, 2).reshape(NQ, C)
         for b in range(B)], axis=0)
    return out.astype(np.float32)
